# revision 1
# baseline (speedup 1.0000x reference)
"""Trainium2 Bass kernel for windowed 3D attention (sparse_attention).

Per window (256 windows on a 16x16 grid): N=294 tokens, d=256, 8 heads x 32.
qkv = x @ w_qkv.T ; A = softmax(q k^T/sqrt(dh) + bias) ; out = (A v) @ w_out.T

Sharding: data-parallel over the grid; core s takes X-rows [2s, 2s+2) = 32 windows.

Device strategy (best measured: 768-789us HW exec, rel err 4.6e-3):
  - all matmul inputs bf16 (host-cast); 1/sqrt(dh) folded into Wq on host
  - Q^T/K^T d-major; V token-major; S^T[j,i] per head, row-packed (K=32) in
    head-groups of 2 over three rotating 2-bank PSUM pools so ScalarE's exp
    trails 3 groups behind the PE (deeper rotation beat bigger exp batches)
  - bias seeded into PSUM by identity matmuls (exact fp32 add), S accumulates
  - A^T = exp(S^T+B^T) straight from PSUM on ScalarE -> bf16 SBUF
  - rowsums via col-packed ones-matmuls -> PE-transpose -> reciprocal [i, head]
  - O^T = V^T A^T col-packed; PE-transpose -> normalize per (head,i) on GPSIMD
    (stride-0 free-dim broadcast of recip) -> PE-transpose back
  - Y = O_norm @ w_out^T token-major -> contiguous DMA out; host reassembles
"""

import os
from contextlib import ExitStack

import numpy as np
import ml_dtypes

import concourse.bass as bass
import concourse.mybir as mybir
import concourse.tile as tile
from concourse import bacc
from concourse.bass_utils import run_bass_kernel_spmd
from concourse.masks import make_identity

F32 = mybir.dt.float32
BF16 = mybir.dt.bfloat16

L, W, D, H = 6, 7, 256, 8
DH = D // H                      # 32
N = L * W * W                    # 294
GX = GY = 16
NCORES = 8
XPC = GX // NCORES               # X-rows per core
NW = XPC * GY                    # 32 windows per core
TOK = NW * N                     # 9408 tokens per core
SCALE = DH ** -0.5
NP = 384                         # N padded to 128 multiple

CH = [(0, 128), (128, 128), (256, 38)]    # j / i chunks

TRACE = False     # set by test.py for profiling runs
_CACHE = {}


def _bcast_free(ap, count):
    """Append a stride-0 innermost free dim of size `count` to an AP."""
    return bass.AP(tensor=ap.tensor, offset=ap.offset, ap=list(ap.ap) + [[0, count]])


def _body(ctx, tc, xT, wqkvT, woutT, biasT, y):
    nc = tc.nc

    const = ctx.enter_context(tc.tile_pool(name="const", bufs=1))
    xpool = ctx.enter_context(tc.tile_pool(name="xin", bufs=4))
    qkpool = ctx.enter_context(tc.tile_pool(name="qk", bufs=3))
    vpool = ctx.enter_context(tc.tile_pool(name="vtok", bufs=3))
    apool = ctx.enter_context(tc.tile_pool(name="at", bufs=6))
    opool = ctx.enter_context(tc.tile_pool(name="ot", bufs=3))
    onpool = ctx.enter_context(tc.tile_pool(name="onorm", bufs=3))
    o2pool = ctx.enter_context(tc.tile_pool(name="ot2", bufs=3))
    ypool = ctx.enter_context(tc.tile_pool(name="yout", bufs=3))
    rspool = ctx.enter_context(tc.tile_pool(name="rs", bufs=3))
    rcpool = ctx.enter_context(tc.tile_pool(name="recip", bufs=3))

    ps_a = ctx.enter_context(tc.tile_pool(name="ps_a", bufs=1, space="PSUM"))
    ps_b = ctx.enter_context(tc.tile_pool(name="ps_b", bufs=1, space="PSUM"))
    ps_c = ctx.enter_context(tc.tile_pool(name="ps_c", bufs=1, space="PSUM"))
    ps_m = ctx.enter_context(tc.tile_pool(name="ps_m", bufs=2, space="PSUM"))

    # ---- resident constants ----
    wqkv_s = const.tile([128, 2, 2 * D], BF16)     # Q^T,K^T weight cols (q pre-scaled)
    nc.sync.dma_start(out=wqkv_s, in_=wqkvT.rearrange("(c p) n -> p c n", c=2)[:, :, 0:2 * D])
    wv_s = const.tile([128, 2, D], BF16)
    nc.sync.dma_start(out=wv_s, in_=wqkvT.rearrange("(c p) n -> p c n", c=2)[:, :, 2 * D:3 * D])
    wout_s = const.tile([128, 2, D], BF16)
    nc.sync.dma_start(out=wout_s, in_=woutT.rearrange("(c p) n -> p c n", c=2))
    bias_s = const.tile([128, 3, H * N], BF16)
    for jc, (j0, jn) in enumerate(CH):
        nc.sync.dma_start(out=bias_s[:jn, jc, :], in_=biasT[j0:j0 + jn, :])
    ident = const.tile([128, 128], F32)
    make_identity(nc, ident)
    ident_b = const.tile([128, 128], BF16)
    make_identity(nc, ident_b)
    ones_b = const.tile([128, 1], BF16)
    nc.vector.memset(ones_b, 1.0)

    for w in range(NW):
        t0 = w * N
        # ---- load x window (d-major, bf16) ----
        xw = xpool.tile([128, 2, N], BF16, tag="xw", name=f"xw{w}")
        nc.sync.dma_start(out=xw, in_=xT.rearrange("(c p) t -> p c t", c=2)[:, :, t0:t0 + N])

        # ---- Q^T / K^T (d-major) ----
        qk = qkpool.tile([128, 4, N], BF16, tag="qk", name=f"qk{w}")
        for m in range(4):
            pq = ps_m.tile([128, 512], F32, tag="psmisc", name=f"pq{w}_{m}")
            for kc in range(2):
                nc.tensor.matmul(
                    pq[:, :N], wqkv_s[:, kc, m * 128:(m + 1) * 128], xw[:, kc, :],
                    start=(kc == 0), stop=(kc == 1))
            nc.vector.tensor_copy(qk[:, m, :], pq[:, :N])

        # ---- V token-major ----
        vtok = vpool.tile([128, 3, D], BF16, tag="vt", name=f"vt{w}")
        for jc, (j0, jn) in enumerate(CH):
            pv = ps_m.tile([128, 512], F32, tag="psmisc", name=f"pv{w}_{jc}")
            for kc in range(2):
                nc.tensor.matmul(
                    pv[:jn, :D], xw[:, kc, j0:j0 + jn], wv_s[:, kc, :],
                    start=(kc == 0), stop=(kc == 1))
            nc.vector.tensor_copy(vtok[:jn, jc, :], pv[:jn, :D])

        # ---- S^T = bias (identity-seeded) + K^T.T @ Q^T ; exp -> A^T bf16 ----
        # head groups of 2 across three rotating 2-bank pools
        at = []
        for jc, (j0, jn) in enumerate(CH):
            a_t = apool.tile([128, H * N], BF16, tag="at", name=f"at{w}_{jc}")
            for gi, heads in enumerate(([0, 1], [2, 3], [4, 5], [6, 7])):
                pool = (ps_a, ps_b, ps_c)[(4 * jc + gi) % 3]
                ps3 = pool.tile([128, 2, 512], F32, tag="s2",
                                name=f"s2_{w}_{jc}_{gi}")
                ng = len(heads)
                for p, h in enumerate(heads):
                    nc.tensor.matmul(
                        ps3[:jn, p, :N], ident_b[:jn, :jn],
                        bias_s[:jn, jc, h * N:(h + 1) * N],
                        start=True, stop=False, skip_group_check=True)
                for p, h in enumerate(heads):
                    nc.tensor.matmul(
                        ps3[:jn, p, :N],
                        qk[32 * (h % 4):32 * (h % 4) + 32, 2 + h // 4, j0:j0 + jn],
                        qk[32 * (h % 4):32 * (h % 4) + 32, h // 4, :],
                        start=False, stop=True, tile_position=(32 * (h % 4), 0),
                        skip_group_check=True)
                nc.scalar.activation(
                    a_t[:jn, heads[0] * N:(heads[-1] + 1) * N].rearrange(
                        "p (c i) -> p c i", c=ng),
                    ps3[:jn, :ng, :N], mybir.ActivationFunctionType.Exp)
            at.append(a_t)

        # ---- rowsums (col-packed ones matmuls, accumulated over j chunks) ----
        prs = [ps_m.tile([128, 512], F32, tag="psmisc", name=f"prs{w}_{g}") for g in range(2)]
        for jc, (j0, jn) in enumerate(CH):
            for g in range(2):
                for c in range(4):
                    h = 4 * g + c
                    nc.tensor.matmul(
                        prs[g][32 * c:32 * c + 1, :N],
                        ones_b[:jn, :], at[jc][:jn, h * N:(h + 1) * N],
                        start=(jc == 0), stop=(jc == 2),
                        tile_position=(0, 32 * c), skip_group_check=True)
        rs_s = rspool.tile([128, 2, N], F32, tag="rcw", name=f"rcw{w}")
        for g in range(2):
            nc.vector.tensor_copy(rs_s[:, g, :], prs[g][:, :N])
        # transpose rowsums to [i, head] and take reciprocals
        rcp = rcpool.tile([128, 3, 8], F32, tag="rcp", name=f"rcp{w}")
        for ic, (i0, isz) in enumerate(CH):
            prt = ps_m.tile([128, 2, 128], F32, tag="psmisc", name=f"prt{w}_{ic}")
            for g in range(2):
                nc.tensor.transpose(prt[:isz, g, :], rs_s[:, g, i0:i0 + isz], ident)
            for g in range(2):
                srcp = prt[:isz, g, :].rearrange("p (c r) -> p c r", r=32)[:, :, 0]
                nc.vector.reciprocal(rcp[:isz, ic, 4 * g:4 * g + 4], srcp)

        # ---- O^T = V^T A^T (col-packed 4 heads) ----
        po = [ps_m.tile([128, 512], F32, tag="psmisc", name=f"po{w}_{g}") for g in range(2)]
        for jc, (j0, jn) in enumerate(CH):
            for g in range(2):
                for c in range(4):
                    h = 4 * g + c
                    nc.tensor.matmul(
                        po[g][32 * c:32 * c + 32, :N],
                        vtok[:jn, jc, 32 * h:32 * h + 32],
                        at[jc][:jn, h * N:(h + 1) * N],
                        start=(jc == 0), stop=(jc == 2),
                        tile_position=(0, 32 * c), skip_group_check=True)
        ot = opool.tile([128, 2, NP], BF16, tag="oraw", name=f"orw{w}")
        for g in range(2):
            nc.vector.tensor_copy(ot[:, g, :N], po[g][:, :N])

        # ---- PE transpose O^T -> O (bf16), normalize, transpose back ----
        onrm = []
        for ic, (i0, isz) in enumerate(CH):
            ptr = ps_m.tile([128, 2, 128], BF16, tag="psmisc", name=f"ptr{w}_{ic}")
            for g in range(2):
                nc.tensor.transpose(ptr[:isz, g, :], ot[:, g, i0:i0 + isz], ident_b)
            onr = onpool.tile([128, 2, 128], BF16, tag="onr", name=f"onr{w}_{ic}")
            nc.vector.tensor_copy(onr[:isz, :, :], ptr[:isz, :, :])
            onm = onpool.tile([128, 2, 128], BF16, tag="onm", name=f"onm{w}_{ic}")
            nc.gpsimd.tensor_tensor(
                out=onm[:isz, :, :].rearrange("p g e -> p (g e)").rearrange("p (h e) -> p h e", h=8),
                in0=onr[:isz, :, :].rearrange("p g e -> p (g e)").rearrange("p (h e) -> p h e", h=8),
                in1=_bcast_free(rcp[:isz, ic, :], 32),
                op=mybir.AluOpType.mult)
            onrm.append(onm)

        ot2 = o2pool.tile([128, 2, NP], BF16, tag="ot2", name=f"ot2_{w}")
        for ic, (i0, isz) in enumerate(CH):
            pt2 = ps_m.tile([128, 2, 128], BF16, tag="psmisc", name=f"pt2_{w}_{ic}")
            for dc in range(2):
                nc.tensor.transpose(pt2[:, dc, :isz], onrm[ic][:isz, dc, :], ident_b[:isz, :isz])
            nc.vector.tensor_copy(ot2[:, :, i0:i0 + isz], pt2[:, :, :isz])

        # ---- Y = O_norm @ w_out^T (token-major) + store ----
        ysb = ypool.tile([128, 3, D], F32, tag="ysb", name=f"ysb{w}")
        for ic, (i0, isz) in enumerate(CH):
            py = ps_m.tile([128, 512], F32, tag="psmisc", name=f"py{w}_{ic}")
            for dc in range(2):
                nc.tensor.matmul(
                    py[:isz, :D], ot2[:, dc, i0:i0 + isz], wout_s[:, dc, :],
                    start=(dc == 0), stop=(dc == 1))
            nc.vector.tensor_copy(ysb[:isz, ic, :], py[:isz, :D])
            nc.scalar.dma_start(out=y[t0 + i0:t0 + i0 + isz, :], in_=ysb[:isz, ic, :])


def _build():
    if "nc" in _CACHE:
        return _CACHE["nc"]
    nc = bacc.Bacc("TRN2", target_bir_lowering=False)
    xT = nc.dram_tensor("xT", [D, TOK], BF16, kind="ExternalInput").ap()
    wqkvT = nc.dram_tensor("wqkvT", [D, 3 * D], BF16, kind="ExternalInput").ap()
    woutT = nc.dram_tensor("woutT", [D, D], BF16, kind="ExternalInput").ap()
    biasT = nc.dram_tensor("biasT", [N, H * N], BF16, kind="ExternalInput").ap()
    y = nc.dram_tensor("y", [TOK, D], F32, kind="ExternalOutput").ap()
    with tile.TileContext(nc) as tc, ExitStack() as ctx:
        _body(ctx, tc, xT, wqkvT, woutT, biasT, y)
    nc.compile()
    _CACHE["nc"] = nc
    return nc


def kernel(x, w_qkv, w_out, bias_table, rel_idx):
    x = np.asarray(x, dtype=np.float32)
    w_qkv = np.asarray(w_qkv, dtype=np.float32)
    w_out = np.asarray(w_out, dtype=np.float32)
    bias_table = np.asarray(bias_table, dtype=np.float32)
    rel_idx = np.asarray(rel_idx)

    # host-side layout prep
    # x[0]: [l, X, Y, w1, w2, d] -> xT [d, (X Y l w1 w2)] bf16
    xt = np.ascontiguousarray(
        x[0].transpose(5, 1, 2, 0, 3, 4)).reshape(D, GX * GY * N).astype(ml_dtypes.bfloat16)
    wq = w_qkv.copy()
    wq[:D] *= SCALE                        # fold attention scale into Wq
    wqkvT = np.ascontiguousarray(wq.T).astype(ml_dtypes.bfloat16)
    woutT = np.ascontiguousarray(w_out.T).astype(ml_dtypes.bfloat16)
    bias = bias_table[rel_idx]             # [i, j, h]
    biasT = np.ascontiguousarray(bias.transpose(1, 2, 0)).reshape(N, H * N).astype(ml_dtypes.bfloat16)

    nc = _build()
    in_maps = []
    for s in range(NCORES):
        xs = np.ascontiguousarray(xt[:, s * TOK:(s + 1) * TOK])
        in_maps.append({"xT": xs, "wqkvT": wqkvT, "woutT": woutT, "biasT": biasT})

    res = run_bass_kernel_spmd(nc, in_maps, core_ids=list(range(NCORES)), trace=TRACE)
    _CACHE["res"] = res
    if TRACE and res.exec_time_ns is not None:
        print(f"HW exec time: {res.exec_time_ns} ns")
        _CACHE["exec_time_ns"] = res.exec_time_ns

    # gather: per-core y [9408, 256] -> [1, l, X, Y, w1, w2, d]
    out = np.empty((1, L, GX, GY, W, W, D), dtype=np.float32)
    for s in range(NCORES):
        yc = res.results[s]["y"].reshape(XPC, GY, L, W, W, D)
        out[0, :, s * XPC:(s + 1) * XPC] = yc.transpose(2, 0, 1, 3, 4, 5)
    return out



# revision 33
# speedup vs baseline: 1.9695x; 1.9695x over previous
"""Trainium2 Bass kernel for windowed 3D attention (sparse_attention).

Per window (256 windows on a 16x16 grid): N=294 tokens, d=256, 8 heads x 32.
qkv = x @ w_qkv.T ; A = softmax(q k^T/sqrt(dh) + bias) ; out = (A v) @ w_out.T
Sharding: data-parallel over the grid; core s takes X-rows [2s, 2s+2) = 32 windows.

v2 design — ~469us HW exec, stable across runs (v1 baseline: ~913-922us),
rel err 5.6e-3.  Startup: x(0)/x(1) DMAs are queued BEFORE the 1.4MB expB
constant so the first QKV isn't stuck behind it (-7us).  SBUF rings (x/qk/v 4-deep, araw 4, at/rr/onorm/ysb 3) are
deliberately deeper than minimal: each extra buffer removes a cross-engine
WAR edge that would otherwise stall the PE and re-throttle the HAM clock.
  - NO identity-seeded bias matmuls (v1 spent ~270us PE on them): bias applied
    as A = exp(S) * expB with expB = exp(bias) precomputed on host (bf16 const
    in SBUF); the multiply runs per 4-head quad: jc0/jc1 on GPSIMD (off the
    DVE queue — its window-tail chain recip/norm/ycast gates PE ring reuse
    and HAM warmth), jc2 on DVE (emitted after the tail chain).
  - NO PE transposes (v1: ~260us): rowsums via ones[jn,32] matmuls whose
    M=32 all-ones stationary lands the row-sum REPLICATED across each head's
    32 partitions ([he, i] layout) -> reciprocal_approx_fast -> normalize O^T
    directly from PSUM on DVE.  Y^T = wout^T.T @ O_norm^T computed d-major;
    host re-transposes the output.
  - S^T tiles are [128, 2, 512] fp32 (2 banks, one head per bank).  HARD HW
    CONSTRAINT (found the painful way): a matmul PSUM output must start at a
    PSUM bank boundary — packing several outputs at free-dim offsets into a
    shared bank aborts the NEFF on device (CoreSim does NOT model this).
  - All S matmuls single-shot (start&stop), K=32 row-packed 4-way via
    tile_position=(32*(h%4), 0); RS/AV K=jn col-packed 4-way at (0, 32*hh).
  - PSUM: S-ring 2x2 banks + misc ring 4x1 bank (qk/v/RS/O/Y) = 8 banks.
  - software-pipelined emission: QKV(w+1) then tail-phase(w-1) pieces
    interleave with window w's 12 S/exp slots.
  - remaining bottleneck: PE ~90% busy but mostly at the cold HAM clock
    (K=4/8, 1.2 GHz; mm dur ~(219+294)/1.2 = 427ns).  Per-window ~0.3-1us
    dependency stalls keep re-throttling the HAM.  Run-to-run variance
    ~460 vs ~550us (HAM phase / thermal bimodality).
"""

import os
from contextlib import ExitStack

import numpy as np
import ml_dtypes

import concourse.bass as bass
import concourse.mybir as mybir
import concourse.tile as tile
from concourse import bacc
from concourse.bass_utils import run_bass_kernel_spmd

F32 = mybir.dt.float32
BF16 = mybir.dt.bfloat16

L, W, D, H = 6, 7, 256, 8
DH = D // H                      # 32
N = L * W * W                    # 294
GX = GY = 16
NCORES = 8
XPC = GX // NCORES               # X-rows per core
NW = int(os.environ.get("KNW", XPC * GY))   # 32 windows per core (overridable for debug)
TOK = NW * N                     # 9408 tokens per core
SCALE = DH ** -0.5

CH = [(0, 128), (128, 128), (256, 38)]    # j / i chunks



TRACE = False     # set by test.py for profiling runs
_CACHE = {}

# ablation flags (debug)
ABL_NO_GPSIMD = bool(int(os.environ.get("ABL_NO_GPSIMD", "0")))
ABL_PAD_S = bool(int(os.environ.get("ABL_PAD_S", "0")))       # 320-elem head stride
ABL_SEQ = bool(int(os.environ.get("ABL_SEQ", "0")))           # no filler interleave
ABL_NORM_SBUF = bool(int(os.environ.get("ABL_NORM_SBUF", "0")))  # copy po->sbuf before norm
ABL_NO_RECIP = bool(int(os.environ.get("ABL_NO_RECIP", "0")))    # tensor_copy instead of recip
KDUMMY = int(os.environ.get("KDUMMY", "0"))  # HAM keep-warm dummy matmuls per boundary
# (measured: dummies run at cold-clock cost but do NOT flip the HAM warm — net loss, keep 0)
STAGE = int(os.environ.get("KSTAGE", "6"))  # 1=QKV 2=+S 3=+exp/mult 4=+RS 5=+AV 6=full


def _body(ctx, tc, xT, wqkvT, woutT, expBT, y):
    nc = tc.nc

    const = ctx.enter_context(tc.tile_pool(name="const", bufs=1))
    xpool = ctx.enter_context(tc.tile_pool(name="xin", bufs=4))
    qkpool = ctx.enter_context(tc.tile_pool(name="qk", bufs=4))
    vpool = ctx.enter_context(tc.tile_pool(name="vtok", bufs=4))
    arawpool = ctx.enter_context(tc.tile_pool(name="araw", bufs=4))
    atpool = ctx.enter_context(tc.tile_pool(name="at", bufs=3))
    rrpool = ctx.enter_context(tc.tile_pool(name="rr", bufs=3))
    onpool = ctx.enter_context(tc.tile_pool(name="onorm", bufs=3))
    ypool = ctx.enter_context(tc.tile_pool(name="ysb", bufs=3))

    # PSUM: S-ring 2 x [128,4,294]f32 (3 banks each) + misc ring 2 x 1 bank
    ps_s = ctx.enter_context(tc.tile_pool(name="ps_s", bufs=2, space="PSUM"))
    ps_m = ctx.enter_context(tc.tile_pool(name="ps_m", bufs=4, space="PSUM"))

    # ---- resident constants ----
    wqkv_s = const.tile([128, 2, 2 * D], BF16)     # Q^T,K^T weight cols (q pre-scaled)
    nc.sync.dma_start(out=wqkv_s, in_=wqkvT.rearrange("(c p) n -> p c n", c=2)[:, :, 0:2 * D])
    wv_s = const.tile([128, 2, D], BF16)
    nc.sync.dma_start(out=wv_s, in_=wqkvT.rearrange("(c p) n -> p c n", c=2)[:, :, 2 * D:3 * D])
    wout_s = const.tile([128, 2, D], BF16)         # w_out^T [he, dout] he-chunked
    nc.sync.dma_start(out=wout_s, in_=woutT.rearrange("(c p) n -> p c n", c=2))
    expb_s = const.tile([128, 3, H * N], BF16)     # exp(B^T)[j, (h,i)], j-chunked
    ones_b = const.tile([128, 32], BF16)
    nc.vector.memset(ones_b, 1.0)

    def load_expb():
        # deferred: 1.4MB const queued AFTER the x(0)/x(1) DMAs so the first
        # window's QKV isn't stuck behind it on the DMA queue (expb is not
        # needed until the first bias-multiply, ~8us in)
        for jc, (j0, jn) in enumerate(CH):
            nc.sync.dma_start(out=expb_s[:jn, jc, :], in_=expBT[j0:j0 + jn, :])

    state = {}   # per-window live tiles
    s_hist = []   # recent S psum tiles (for HAM keep-warm dummy matmuls)

    def pe_dummy():
        # dependency-free matmul overwriting an already-consumed S-tile region
        # (slot 0, offset 0 = bank-aligned): keeps the PE_HAM activity window
        # busy across short dependency stalls so real matmuls run at 2.4 GHz.
        if len(s_hist) < 2 or not KDUMMY:
            return
        t = s_hist[-2]
        for _ in range(KDUMMY):
            nc.tensor.matmul(
                t[0:32, 0, 0:32], ones_b[:32, :32], ones_b[:32, :32],
                start=True, stop=True, skip_group_check=True)

    def load_x(w):
        t0 = w * N
        xw = xpool.tile([128, 2, N], BF16, tag="xw", name=f"xw{w}")
        nc.sync.dma_start(out=xw, in_=xT.rearrange("(c p) t -> p c t", c=2)[:, :, t0:t0 + N])
        state[("x", w)] = xw

    def qkv_pieces(w):
        """7 closures: QKV projection of window w, one psum tile each."""
        xw = state[("x", w)]
        qk_sb = qkpool.tile([128, 4, N], BF16, tag="qk", name=f"qk{w}")
        vtok = vpool.tile([128, 3, D], BF16, tag="vt", name=f"vt{w}")
        state[("qk", w)] = qk_sb
        state[("v", w)] = vtok

        def qk_piece(m):
            def run():
                pq = ps_m.tile([128, 512], F32, tag="ring", name=f"pq{w}_{m}")
                for kc in range(2):
                    nc.tensor.matmul(
                        pq[:, :N], wqkv_s[:, kc, m * 128:(m + 1) * 128], xw[:, kc, :],
                        start=(kc == 0), stop=(kc == 1))
                nc.vector.tensor_copy(qk_sb[:, m, :], pq[:, :N])
            return run

        def v_piece(jc):
            def run():
                j0, jn = CH[jc]
                pv = ps_m.tile([128, 512], F32, tag="ring", name=f"pv{w}_{jc}")
                for kc in range(2):
                    nc.tensor.matmul(
                        pv[:jn, :D], xw[:, kc, j0:j0 + jn], wv_s[:, kc, :],
                        start=(kc == 0), stop=(kc == 1))
                nc.vector.tensor_copy(vtok[:jn, jc, :], pv[:jn, :D])
            return run

        return [qk_piece(m) for m in range(4)] + [v_piece(jc) for jc in range(3)]

    def tail_pieces(w):
        """6 closures: rowsums+recip / AV+normalize / out-proj of window w."""
        at = state[("at", w)]
        vtok = state[("v", w)]
        rr = rrpool.tile([128, 2, N], F32, tag="rr", name=f"rr{w}")
        onrm = onpool.tile([128, 2, N], BF16, tag="onrm", name=f"on{w}")
        ysb = ypool.tile([128, 2, N], F32, tag="ysb", name=f"ysb{w}")
        t0 = w * N

        def rs_piece(g):
            def run():
                prs = ps_m.tile([128, 512], F32, tag="ring", name=f"prs{w}_{g}")
                for jc, (j0, jn) in enumerate(CH):
                    for hh in range(4):
                        h = 4 * g + hh
                        nc.tensor.matmul(
                            prs[32 * hh:32 * hh + 32, :N],
                            ones_b[:jn, :], at[:jn, jc, h, :],
                            start=(jc == 0), stop=(jc == 2),
                            tile_position=(0, 32 * hh), skip_group_check=True)
                if ABL_NO_RECIP:
                    nc.vector.tensor_copy(rr[:, g, :], prs[:, :N])
                else:
                    nc.vector.reciprocal_approx_fast(out=rr[:, g, :], in_=prs[:, :N])
            return run

        def av_piece(g):
            def run():
                po = ps_m.tile([128, 512], F32, tag="ring", name=f"po{w}_{g}")
                for jc, (j0, jn) in enumerate(CH):
                    for hh in range(4):
                        h = 4 * g + hh
                        nc.tensor.matmul(
                            po[32 * hh:32 * hh + 32, :N],
                            vtok[:jn, jc, 32 * h:32 * h + 32], at[:jn, jc, h, :],
                            start=(jc == 0), stop=(jc == 2),
                            tile_position=(0, 32 * hh), skip_group_check=True)
                if ABL_NORM_SBUF:
                    osb = onpool.tile([128, N], F32, tag="osb", name=f"osb{w}_{g}")
                    nc.vector.tensor_copy(osb, po[:, :N])
                    nc.vector.tensor_tensor(
                        out=onrm[:, g, :], in0=osb, in1=rr[:, g, :],
                        op=mybir.AluOpType.mult)
                else:
                    nc.vector.tensor_tensor(
                        out=onrm[:, g, :], in0=po[:, :N], in1=rr[:, g, :],
                        op=mybir.AluOpType.mult)
            return run

        def y_piece(d):
            def run():
                py = ps_m.tile([128, 512], F32, tag="ring", name=f"py{w}_{d}")
                for g in range(2):
                    nc.tensor.matmul(
                        py[:, :N], wout_s[:, g, 128 * d:128 * (d + 1)], onrm[:, g, :],
                        start=(g == 0), stop=(g == 1))
                nc.vector.tensor_copy(ysb[:, d, :], py[:, :N])
                nc.sync.dma_start(out=y[128 * d:128 * (d + 1), t0:t0 + N], in_=ysb[:, d, :])
            return run

        pieces = []
        if STAGE >= 4:
            pieces += [rs_piece(0), rs_piece(1)]
        if STAGE >= 5:
            pieces += [av_piece(0), av_piece(1)]
        if STAGE >= 6:
            pieces += [y_piece(0), y_piece(1)]
        return pieces

    def s_slot(w, jc, g2, qd_tiles):
        """S matmuls + exp for heads (2*g2, 2*g2+1) of (window w, j-chunk jc),
        then (after the second tile of a quad) the bias multiply."""
        if STAGE < 2:
            return
        qk_sb = state[("qk", w)]
        at = state[("at", w)]
        j0, jn = CH[jc]
        s_t = ps_s.tile([128, 2, 512], F32, tag="sring", name=f"s{w}_{jc}_{g2}")
        s_hist.append(s_t)
        if len(s_hist) > 3:
            s_hist.pop(0)
        for p in range(2):
            h = 2 * g2 + p
            hh = h % 4
            g = h // 4
            nc.tensor.matmul(
                s_t[:jn, p, :N],
                qk_sb[32 * hh:32 * hh + 32, 2 + g, j0:j0 + jn],   # K^T [32, jn]
                qk_sb[32 * hh:32 * hh + 32, g, :],                # Q^T [32, 294]
                start=True, stop=True, skip_group_check=True,
                tile_position=(32 * hh, 0))
        if STAGE < 3:
            nc.vector.tensor_copy(at[:jn, jc, 2 * g2:2 * g2 + 2, :], s_t[:jn, :, :N])
            return
        # quad = two consecutive 2-head tiles share one araw tile so the
        # bias multiply runs as one [jn, 4, 294] op
        qd = g2 // 2
        sub = g2 % 2
        if sub == 0:
            qd_tiles[qd] = arawpool.tile(
                [128, 4, N], BF16, tag="araw", name=f"ar{w}_{jc}_{qd}")
        araw = qd_tiles[qd]
        nc.scalar.activation(
            araw[:jn, 2 * sub:2 * sub + 2, :], s_t[:jn, :, :N],
            mybir.ActivationFunctionType.Exp)
        if sub == 1:
            # bias multiply for the whole quad (heads 4*qd .. 4*qd+4)
            eng = nc.gpsimd if (jc < 2 and not ABL_NO_GPSIMD) else nc.vector
            eng.tensor_tensor(
                out=at[:jn, jc, 4 * qd:4 * qd + 4, :],
                in0=araw[:jn, :, :],
                in1=expb_s[:jn, jc, 4 * qd * N:(4 * qd + 4) * N].rearrange(
                    "p (h i) -> p h i", h=4),
                op=mybir.AluOpType.mult)

    # ---- prologue ----
    load_x(0)
    load_x(1)
    load_expb()
    fillers = qkv_pieces(0)
    for f in fillers:
        f()

    # ---- pipelined window loop ----
    for w in range(NW):
        if w + 2 < NW:
            load_x(w + 2)
        state[("at", w)] = atpool.tile([128, 3, H, N], BF16, tag="at", name=f"at{w}")

        fillers = []
        if w + 1 < NW:
            fillers += qkv_pieces(w + 1)
        if w > 0:
            fillers += tail_pieces(w - 1)

        # interleave ~1 filler piece per S slot to keep PE fed without
        # delaying the next S tile behind a long PE burst
        nslots = 12
        total = len(fillers)
        taken = 0
        for si, (jc, g2) in enumerate([(j, g) for j in range(3) for g in range(4)]):
            if g2 == 0:
                qd_tiles = {}
            s_slot(w, jc, g2, qd_tiles)
            pe_dummy()
            if not ABL_SEQ:
                want = total * (si + 1) // nslots
                while taken < want:
                    fillers[taken]()
                    pe_dummy()
                    taken += 1
        while taken < total:
            fillers[taken]()
            pe_dummy()
            taken += 1

    # final tail
    for f in tail_pieces(NW - 1):
        f()


def _build():
    if "nc" in _CACHE:
        return _CACHE["nc"]
    nc = bacc.Bacc("TRN2", target_bir_lowering=False)
    xT = nc.dram_tensor("xT", [D, TOK], BF16, kind="ExternalInput").ap()
    wqkvT = nc.dram_tensor("wqkvT", [D, 3 * D], BF16, kind="ExternalInput").ap()
    woutT = nc.dram_tensor("woutT", [D, D], BF16, kind="ExternalInput").ap()
    expBT = nc.dram_tensor("expBT", [N, H * N], BF16, kind="ExternalInput").ap()
    y = nc.dram_tensor("y", [D, TOK], F32, kind="ExternalOutput").ap()
    with tile.TileContext(nc) as tc, ExitStack() as ctx:
        _body(ctx, tc, xT, wqkvT, woutT, expBT, y)
    nc.compile()
    _CACHE["nc"] = nc
    return nc


def kernel(x, w_qkv, w_out, bias_table, rel_idx):
    x = np.asarray(x, dtype=np.float32)
    w_qkv = np.asarray(w_qkv, dtype=np.float32)
    w_out = np.asarray(w_out, dtype=np.float32)
    bias_table = np.asarray(bias_table, dtype=np.float32)
    rel_idx = np.asarray(rel_idx)

    # host-side layout prep
    # x[0]: [l, X, Y, w1, w2, d] -> xT [d, (X Y l w1 w2)] bf16
    xt = np.ascontiguousarray(
        x[0].transpose(5, 1, 2, 0, 3, 4)).reshape(D, GX * GY * N).astype(ml_dtypes.bfloat16)
    wq = w_qkv.copy()
    wq[:D] *= SCALE                        # fold attention scale into Wq
    wqkvT = np.ascontiguousarray(wq.T).astype(ml_dtypes.bfloat16)
    woutT = np.ascontiguousarray(w_out.T).astype(ml_dtypes.bfloat16)
    bias = bias_table[rel_idx]             # [i, j, h]
    expBT = np.ascontiguousarray(
        np.exp(bias.transpose(1, 2, 0))).reshape(N, H * N).astype(ml_dtypes.bfloat16)

    nc = _build()
    in_maps = []
    for s in range(NCORES):
        xs = np.ascontiguousarray(xt[:, s * TOK:(s + 1) * TOK])
        in_maps.append({"xT": xs, "wqkvT": wqkvT, "woutT": woutT, "expBT": expBT})

    res = run_bass_kernel_spmd(nc, in_maps, core_ids=list(range(NCORES)), trace=TRACE)
    _CACHE["res"] = res
    if TRACE and res.exec_time_ns is not None:
        print(f"HW exec time: {res.exec_time_ns} ns")
        _CACHE["exec_time_ns"] = res.exec_time_ns

    # gather: per-core y [256, 9408] d-major -> [1, l, X, Y, w1, w2, d]
    out = np.empty((1, L, GX, GY, W, W, D), dtype=np.float32)
    for s in range(NCORES):
        yc = res.results[s]["y"].reshape(D, XPC, GY, L, W, W)
        out[0, :, s * XPC:(s + 1) * XPC] = yc.transpose(3, 1, 2, 4, 5, 0)
    return out
